# revision 41
# baseline (speedup 1.0000x reference)
"""Causal GQA attention (B=2, S=2048, HID=2048, H=16, KVH=4, D=128) on 8 TRN2 cores.

Sharding: core c -> batch c//4, kv-group c%4 (4 q-heads + 1 kv-head).
o_proj is row-split by head group; host sums the 4 partials per batch.

Device kernel (per core, bf16 matmuls / f32 accumulation):
  xT (host-pretransposed [HID, S]) -> qT/kT/vT projections -> RoPE (pair-permuted
  on host into wq/wk columns, applied via half-swap + cos/sin muls straight from
  PSUM) -> scores^T = K.Q^T per [128k, 512q] tile; j=1 diagonal tiles get the
  causal -200 upper-tri added in-PSUM via a second matmul -> exp on ScalarE ->
  AV^T with V-natural (vN built by DMA transpose) -> denominators via bf16
  VectorE accumulation + GpSimd partition_all_reduce -> normalize -> o_proj
  (out[t, o] += avN_h.T @ wo_h), units interleaved through the j=1 phase.
Diagonal supertiles only compute the valid q-range [128r:512].
"""

import numpy as np
import ml_dtypes

BF16 = ml_dtypes.bfloat16

B, S, HID = 2, 2048, 2048
H, KVH, D = 16, 4, 128
P = 128
KO = HID // P          # 16 contraction tiles
HQ = H // KVH          # 4 q heads per core
NTB = S // 512         # 4 token tiles of 512 (projection)
NQB = S // 512         # 4 query blocks of 512 (attention)
QW = 512
NKB = S // P           # 16 key blocks of 128
N_CORES = 8

_CACHE = {}


def _build_nc():
    import concourse.tile as tile
    from concourse import bacc, mybir
    from concourse.masks import make_identity
    from contextlib import ExitStack

    bf = mybir.dt.bfloat16
    f32 = mybir.dt.float32
    AF = mybir.ActivationFunctionType
    QS = 2 * QW  # 1024-wide scores/exp supertile

    nc = bacc.Bacc("TRN2", target_bir_lowering=False, debug=False,
                   num_devices=N_CORES)

    xT_d = nc.dram_tensor("xT", [HID, S], bf, kind="ExternalInput").ap()
    wq_d = nc.dram_tensor("wq", [P, KO * HQ * D], bf, kind="ExternalInput").ap()
    wk_d = nc.dram_tensor("wk", [P, KO * D], bf, kind="ExternalInput").ap()
    wv_d = nc.dram_tensor("wv", [P, KO * D], bf, kind="ExternalInput").ap()
    wo_d = nc.dram_tensor("wo", [P, HQ * HID], bf, kind="ExternalInput").ap()
    cs_d = nc.dram_tensor("cs2", [P, S], bf, kind="ExternalInput").ap()
    ss_d = nc.dram_tensor("ss2", [P, S], bf, kind="ExternalInput").ap()
    tn_d = nc.dram_tensor("trineg", [P, P], bf, kind="ExternalInput").ap()
    out_d = nc.dram_tensor("out", [S, HID], bf, kind="ExternalOutput").ap()
    out_r = out_d.rearrange("(tb p) o -> p tb o", p=P)

    import concourse.bass_isa as bass_isa

    with tile.TileContext(nc) as tc:
        with ExitStack() as octx:
            const = octx.enter_context(tc.tile_pool(name="const", bufs=1))
            rope_p = octx.enter_context(tc.tile_pool(name="rope", bufs=2))
            at_p = octx.enter_context(tc.tile_pool(name="at", bufs=8))
            acc_p = octx.enter_context(tc.tile_pool(name="acc", bufs=4))
            small = octx.enter_context(tc.tile_pool(name="small", bufs=2))
            ost_p = octx.enter_context(tc.tile_pool(name="ost", bufs=4))
            # xT stays resident for the whole kernel: closing its pool would
            # emit a cross-engine barrier that stalls the PE on the last
            # projection's rope chain (~4.5us)
            xt_pool = octx.enter_context(tc.tile_pool(name="xt", bufs=1))

            # ---- warmup constants first so the PE can start the moment the
            # runtime releases the engines ----
            ident = const.tile([P, P], bf, tag="ident", name="ident")
            make_identity(nc, ident[:])
            idp = const.tile([P, QW], bf, tag="idp", name="idp")
            nc.vector.memset(idp[:], 0.0)
            nc.scalar.copy(idp[:, 0:P], ident[:])
            ones = const.tile([P, 1], bf, tag="ones", name="ones")
            nc.vector.memset(ones[:], 1.0)

            # ---- persistent loads, xT prioritized (v-proj ko-outer consumes
            # chunks as they land; weights for later phases trail) ----
            xTk = [xt_pool.tile([P, S], bf, tag=f"xT{ko}", name=f"xT{ko}")
                   for ko in range(KO)]
            # first chunk split into quarters so the first projection matmul
            # can start the moment 512 tokens have landed
            for i4 in range(4):
                nc.sync.dma_start(xTk[0][:, i4 * QW:(i4 + 1) * QW],
                                  xT_d[0:P, i4 * QW:(i4 + 1) * QW])
            # wv/wk ko=0 slices land first (tiny) so the first projection
            # matmuls can issue ~5us earlier than waiting the full weights
            wv_sb = const.tile([P, KO, D], bf, tag="wv", name="wv")
            wv_r2 = wv_d.rearrange("p (ko n) -> p ko n", ko=KO)
            nc.sync.dma_start(wv_sb[:, 0:1, :], wv_r2[:, 0:1, :])
            wk_sb = const.tile([P, KO, D], bf, tag="wk", name="wk")
            wk_r2 = wk_d.rearrange("p (ko n) -> p ko n", ko=KO)
            nc.sync.dma_start(wk_sb[:, 0:1, :], wk_r2[:, 0:1, :])
            nc.sync.dma_start(wv_sb[:, 1:KO, :], wv_r2[:, 1:KO, :])
            nc.sync.dma_start(wk_sb[:, 1:KO, :], wk_r2[:, 1:KO, :])
            for ko in range(1, 8):
                nc.sync.dma_start(xTk[ko][:], xT_d[ko * P:(ko + 1) * P, :])
            wq_sb = const.tile([P, KO, HQ * D], bf, tag="wq", name="wq")
            nc.sync.dma_start(wq_sb[:], wq_d.rearrange("p (ko n) -> p ko n", ko=KO))
            for ko in range(8, KO):
                nc.sync.dma_start(xTk[ko][:], xT_d[ko * P:(ko + 1) * P, :])
            cs_sb = const.tile([P, S], bf, tag="cs", name="cs")
            nc.sync.dma_start(cs_sb[:], cs_d[:])
            ss_sb = const.tile([P, S], bf, tag="ss", name="ss")
            nc.sync.dma_start(ss_sb[:], ss_d[:])
            tn_sb = const.tile([P, P], bf, tag="tn", name="tn")
            nc.sync.dma_start(tn_sb[:], tn_d[:])
            wo_sb = const.tile([P, HQ, HID], bf, tag="wo", name="wo")
            nc.sync.dma_start(wo_sb[:], wo_d.rearrange("p (h o) -> p h o", h=HQ))

            qR = [const.tile([P, S], bf, tag=f"qR{h}", name=f"qR{h}")
                  for h in range(HQ)]
            kR = const.tile([P, S], bf, tag="kR", name="kR")
            vT_sb = const.tile([P, S], bf, tag="vT", name="vT")
            vN = const.tile([P, NKB, D], bf, tag="vN", name="vN")
            avN = [const.tile([P, S], bf, tag=f"avN{h}", name=f"avN{h}")
                   for h in range(HQ)]

            def rope_tile(ps, out_sl, tb):
                tsl = slice(tb * QW, (tb + 1) * QW)
                # half-swap via Scalar partition-offset copies straight from
                # PSUM (no intermediate bf16 copy of the raw tile)
                sw = rope_p.tile([P, QW], bf, tag="rp_sw", name="rp_sw")
                nc.scalar.copy(sw[0:64, :], ps[64:128, :])
                nc.scalar.copy(sw[64:128, :], ps[0:64, :])
                t1 = rope_p.tile([P, QW], bf, tag="rp_t1", name="rp_t1")
                nc.vector.tensor_mul(t1[:], ps[:], cs_sb[:, tsl])
                t2 = rope_p.tile([P, QW], bf, tag="rp_t2", name="rp_t2")
                nc.vector.tensor_mul(t2[:], sw[:], ss_sb[:, tsl])
                nc.vector.tensor_add(out_sl, t1[:], t2[:])

            # ---- PSUM pools for the WHOLE kernel: no scope transitions, so
            # no released-zone overlap barriers between phases.
            # ps_s: 2x[P,1024] (4 banks), ps_av: 2x[P,512], op_il: 2x[P,512].
            if True:
                ps_s_p = octx.enter_context(
                    tc.tile_pool(name="ps_s", bufs=2, space="PSUM"))
                ps_av_p = octx.enter_context(
                    tc.tile_pool(name="ps_av", bufs=2, space="PSUM"))
                op_il_p = octx.enter_context(
                    tc.tile_pool(name="op_il", bufs=2, space="PSUM"))

                def s_tile():
                    return ps_s_p.tile([P, QS], f32, tag="s", name="s")

                def av_tile():
                    return ps_av_p.tile([P, QW], f32, tag="av", name="av")

                def o_tile():
                    return op_il_p.tile([P, QW], f32, tag="o", name="o")

                # ---- HAM warmup: standalone weight loads keep the PE clock
                # ramping without touching PSUM ----
                for _ in range(45):
                    nc.tensor.ldweights(ident[:])

                # ---- v+k projections in one ko-outer sweep; v uses the two
                # [P,1024] score tiles, k the four [P,512] av/o tiles.
                # ldweights fillers absorb xT chunk-arrival jitter without
                # letting the PE clock gate drop. ----
                ta_v, tb_v = s_tile(), s_tile()
                qv = [ta_v[:, 0:QW], ta_v[:, QW:QS],
                      tb_v[:, 0:QW], tb_v[:, QW:QS]]
                qk = [av_tile(), av_tile(), o_tile(), o_tile()]
                for ko in range(KO):
                    for i in range(4):
                        nc.tensor.matmul(
                            qv[i][:D, :], lhsT=wv_sb[:, ko, 0:D],
                            rhs=xTk[ko][:, i * QW:(i + 1) * QW],
                            start=(ko == 0), stop=(ko == KO - 1))
                    for i in range(4):
                        nc.tensor.matmul(
                            qk[i][:D, :], lhsT=wk_sb[:, ko, 0:D],
                            rhs=xTk[ko][:, i * QW:(i + 1) * QW],
                            start=(ko == 0), stop=(ko == KO - 1))
                    if ko < 12:
                        for _ in range(6 if ko < 6 else 4):
                            nc.tensor.ldweights(ident[:])
                for tb in range(NTB):
                    nc.scalar.copy(vT_sb[:, tb * QW:(tb + 1) * QW], qv[tb][:])
                for tb in range(NTB):
                    rope_tile(qk[tb][:], kR[:, tb * QW:(tb + 1) * QW], tb)
                # ---- v transpose to natural layout via DMA xbar ----
                for kb in range(NKB):
                    nc.sync.dma_start_transpose(
                        vN[:, kb, :], vT_sb[:, kb * P:(kb + 1) * P])

                def q_proj_head(h, tile_fn):
                    for pair in range(2):
                        t = tile_fn()
                        halves = [t[:, 0:QW], t[:, QW:QS]]
                        for ko in range(KO):
                            for i in range(2):
                                tb = 2 * pair + i
                                nc.tensor.matmul(
                                    halves[i][:D, :],
                                    lhsT=wq_sb[:, ko, h * D:(h + 1) * D],
                                    rhs=xTk[ko][:, tb * QW:(tb + 1) * QW],
                                    start=(ko == 0), stop=(ko == KO - 1))
                        for i in range(2):
                            tb = 2 * pair + i
                            rope_tile(halves[i],
                                      qR[h][:, tb * QW:(tb + 1) * QW], tb)

                for h in range(HQ - 1):
                    q_proj_head(h, s_tile)

                # head 3's q projection borrows the av/o_proj rings so the
                # ps_s ring's pending rope reads (h2) drain behind it; its
                # two pairs are emitted around the first two score supertiles
                h3 = HQ - 1

                def h3_proj_pair(pair):
                    hv = [av_tile(), av_tile()] if pair == 0 else \
                         [o_tile(), o_tile()]
                    for ko in range(KO):
                        for i in range(2):
                            tb = 2 * pair + i
                            nc.tensor.matmul(
                                hv[i][:D, :],
                                lhsT=wq_sb[:, ko, h3 * D:(h3 + 1) * D],
                                rhs=xTk[ko][:, tb * QW:(tb + 1) * QW],
                                start=(ko == 0), stop=(ko == KO - 1))
                    for i in range(2):
                        tb = 2 * pair + i
                        rope_tile(hv[i][:], qR[h3][:, tb * QW:(tb + 1) * QW],
                                  tb)

                seq = [(h, j, kb)
                       for j in range(2)
                       for h in range(HQ)
                       for kb in range(8 * j + 8)]
                st = {}
                # o_proj units: (tb, ob) -> 4 hh-accumulated [P,512] matmuls.
                # Units for tb<8 become ready once all j=0 groups finish; they
                # are interleaved throughout the j=1 phase to fill PE stalls.
                uq = [(tb, ob) for tb in range(8) for ob in range(4)]
                drain_flip = [0]

                def oproj_unit(tb, ob, tile_fn):
                    pso = tile_fn()
                    for hh in range(HQ):
                        nc.tensor.matmul(
                            pso[:], lhsT=avN[hh][:, tb * P:(tb + 1) * P],
                            rhs=wo_sb[:, hh, ob * QW:(ob + 1) * QW],
                            start=(hh == 0), stop=(hh == HQ - 1))
                    ot = ost_p.tile([P, QW], bf, tag="ot", name="ot")
                    if drain_flip[0] % 2:
                        nc.vector.tensor_copy(ot[:], pso[:])
                    else:
                        nc.scalar.copy(ot[:], pso[:])
                    drain_flip[0] += 1
                    nc.sync.dma_start(
                        out_r[:, tb, ob * QW:(ob + 1) * QW], ot[:])

                def masked_group(ps_s, c0, c1, kb, h, q0):
                    # causal mask lands in-PSUM: -200 upper-tri via a second
                    # matmul in the same accumulation group
                    nc.tensor.matmul(
                        ps_s[:, c0:c1], lhsT=tn_sb[:],
                        rhs=idp[:, 0:c1 - c0],
                        start=True, stop=False)
                    nc.tensor.matmul(
                        ps_s[:, c0:c1],
                        lhsT=kR[:, kb * P:(kb + 1) * P],
                        rhs=qR[h][:, q0 + c0:q0 + c1],
                        start=False, stop=True)

                def scores_i(h, j, kb):
                    q0 = j * QS
                    if kb == 0:
                        st[(h, j)] = {
                            "av": [ps_av_p.tile([P, QW], f32, tag="av",
                                                name="av") for _ in range(2)],
                            "acc": [],
                            "ats": {},
                        }
                    s = st[(h, j)]
                    r = kb - 8 * j
                    lo = 128 * r if r >= 0 else 0
                    ps_s = s_tile()
                    if lo < QW:
                        if r >= 0:
                            masked_group(ps_s, lo, QW, kb, h, q0)
                        else:
                            nc.tensor.matmul(
                                ps_s[:, lo:QW],
                                lhsT=kR[:, kb * P:(kb + 1) * P],
                                rhs=qR[h][:, q0 + lo:q0 + QW],
                                start=True, stop=True)
                    l1 = max(lo, QW)
                    if r >= 0 and lo >= QW:
                        masked_group(ps_s, l1, QS, kb, h, q0)
                    else:
                        nc.tensor.matmul(
                            ps_s[:, l1:QS],
                            lhsT=kR[:, kb * P:(kb + 1) * P],
                            rhs=qR[h][:, q0 + l1:q0 + QS],
                            start=True, stop=True)
                    at = at_p.tile([P, QS], bf, tag="at", name="at")
                    nc.scalar.activation(at[:, lo:QS], ps_s[:, lo:QS], AF.Exp)
                    s["ats"][kb] = at

                def half_epi(h, j, half):
                    # denominator + normalize for one 512-query half; half 0
                    # is complete (causally) already at kb == 8j+3, freeing
                    # its PSUM accumulator early
                    s = st[(h, j)]
                    q0 = j * QS
                    accs = s["acc"]
                    # dn shares the o_proj PSUM ring (tag "o") to stay within
                    # the 8-bank budget
                    dn_t = op_il_p.tile([P, QW], f32, tag="o", name="dn")
                    for aj, a in enumerate(accs):
                        nc.tensor.matmul(
                            dn_t[0:1, :], lhsT=ones[:],
                            rhs=a[:, half * QW:(half + 1) * QW],
                            start=(aj == 0), stop=(aj == len(accs) - 1))
                    recip = small.tile([1, QW], f32, tag="recip",
                                       name="recip")
                    nc.vector.reciprocal_approx_fast(recip[:], dn_t[0:1, :])
                    rb = small.tile([P, QW], f32, tag="rb", name="rb")
                    nc.gpsimd.partition_broadcast(rb[:], recip[:])
                    nc.vector.tensor_mul(
                        avN[h][:, q0 + half * QW:q0 + (half + 1) * QW],
                        s["av"][half][:], rb[:])

                epiq = []

                def accum_i(h, j, kb):
                    nkb = 8 * j + 8
                    s = st[(h, j)]
                    ps_av = s["av"]
                    r = kb - 8 * j
                    lo = 128 * r if r >= 0 else 0
                    at = s["ats"][kb]
                    l1 = max(lo, QW)
                    half0 = kb <= 8 * j + 3
                    if half0:  # av pair shares lhsT=vN[kb]
                        nc.tensor.matmul(
                            ps_av[0][:, lo:QW], lhsT=vN[:, kb, :],
                            rhs=at[:, lo:QW],
                            start=(kb == 0), stop=(kb == 8 * j + 3))
                    # av[1]'s first writes are deferred to kb=4 so the
                    # previous head's half1 normalize chain has drained by
                    # the time the slot's start=True write issues (kills the
                    # per-head av-recycle stall)
                    if kb < 4:
                        s.setdefault("av1_defer", []).append(
                            (kb, at, l1))
                    else:
                        if kb == 4:
                            nc.tensor.matmul(
                                ps_av[1][:, l1 - QW:QW], lhsT=vN[:, kb, :],
                                rhs=at[:, l1:QS],
                                start=True, stop=False)
                            for dkb, dat, dl1 in s.pop("av1_defer"):
                                nc.tensor.matmul(
                                    ps_av[1][:, dl1 - QW:QW],
                                    lhsT=vN[:, dkb, :],
                                    rhs=dat[:, dl1:QS],
                                    start=False, stop=False)
                        else:
                            nc.tensor.matmul(
                                ps_av[1][:, l1 - QW:QW], lhsT=vN[:, kb, :],
                                rhs=at[:, l1:QS],
                                start=False, stop=(kb == nkb - 1))
                    # denominator partial sums accumulate on VectorE in bf16
                    # (2x mode); j=1 alternates two accumulators to halve
                    # rounding drift.  Initialize via at+at adds (no copies).
                    accs = s["acc"]
                    nacc = 1 + j
                    if kb < 2 * nacc:
                        if kb % 2 == 1:  # (0,1)->acc0, j=1 also: (2,3)->acc1
                            acc = acc_p.tile([P, QS], bf, tag="acc",
                                             name="acc")
                            accs.append(acc)
                            pv = s["ats"][kb - 1]
                            rp = kb - 1 - 8 * j
                            plo = 128 * rp if rp >= 0 else 0
                            nc.vector.tensor_add(acc[:, lo:QS],
                                                 pv[:, lo:QS],
                                                 at[:, lo:QS])
                            if lo > plo:
                                nc.vector.tensor_copy(acc[:, plo:lo],
                                                      pv[:, plo:lo])
                            s["ats"].pop(kb - 1)
                            s["ats"].pop(kb)
                    else:
                        acc = accs[kb % nacc]
                        nc.vector.tensor_add(acc[:, lo:QS], acc[:, lo:QS],
                                             at[:, lo:QS])
                        s["ats"].pop(kb)
                    # fill PE stalls in the j=1 phase with ready o_proj units
                    if j == 1 and kb in (2, 10) and uq:
                        tb_o, ob_o = uq.pop(0)
                        oproj_unit(tb_o, ob_o, o_tile)
                    if kb == 8 * j + 3:
                        epiq.append((h, j, 0))
                    if kb == nkb - 1:
                        epiq.append((h, j, 1))

                def flush_one():
                    he, je, hf = epiq.pop(0)
                    half_epi(he, je, hf)
                    if hf == 1:
                        del st[(he, je)]
                    # bridge epilogue-chain latency at phase/head boundaries
                    # with ready o_proj units (tb<4 ready after all j=0 half0
                    # epilogues; the rest after j=0 completes)
                    npop = 0
                    if hf == 0 and he == HQ - 1 and je == 0:
                        npop = 4
                    elif hf == 1 and he == HQ - 1 and je == 0:
                        npop = 3
                    elif hf == 0 and je == 1:
                        npop = 2
                    elif hf == 1 and je == 1:
                        npop = 2
                    for _ in range(npop):
                        if uq:
                            tb_o, ob_o = uq.pop(0)
                            oproj_unit(tb_o, ob_o, o_tile)

                # 2-deep scores lookahead: AV(i) waits exp(i), so keep two
                # score supertiles in flight to hide the exp chain latency.
                # Epilogues flush after the following scores emission, always
                # before the next accum (av-slot write-after-read ordering).
                h3_proj_pair(0)
                scores_i(*seq[0])
                scores_i(*seq[1])
                h3_proj_pair(1)
                for i in range(2, len(seq)):
                    scores_i(*seq[i])
                    while epiq:
                        flush_one()
                    accum_i(*seq[i - 2])
                while epiq:
                    flush_one()
                accum_i(*seq[-2])
                while epiq:
                    flush_one()
                accum_i(*seq[-1])
                while epiq:
                    flush_one()

                # ---- o_proj tail: remaining units alternate the av/o rings
                # (effective 4-deep), drains alternating Scalar/Vector ----
                rest = uq + [(tb, ob) for tb in range(8, NKB)
                             for ob in range(4)]
                for n, (tb, ob) in enumerate(rest):
                    oproj_unit(tb, ob, av_tile if n % 2 else o_tile)

    nc.compile()
    return nc


def _prep_inputs(x, freqs_cis, wq, wk, wv, wo):
    x = np.asarray(x, dtype=np.float32)
    freqs = np.asarray(freqs_cis, dtype=np.float32)
    wq = np.asarray(wq, dtype=np.float32)
    wk = np.asarray(wk, dtype=np.float32)
    wv = np.asarray(wv, dtype=np.float32)
    wo = np.asarray(wo, dtype=np.float32)

    perm = np.concatenate([np.arange(0, D, 2), np.arange(1, D, 2)])
    cos = freqs[..., 0].T.astype(np.float32)            # [64, S]
    sin = freqs[..., 1].T.astype(np.float32)
    cs2 = np.ascontiguousarray(np.concatenate([cos, cos], 0)).astype(BF16)
    ss2 = np.ascontiguousarray(np.concatenate([-sin, sin], 0)).astype(BF16)

    wq_p = (wq.reshape(HID, H, D)[:, :, perm] * D**-0.5).astype(BF16)
    wk_p = wk.reshape(HID, KVH, D)[:, :, perm].astype(BF16)
    wv_r = wv.reshape(HID, KVH, D).astype(BF16)
    wo_r = wo.reshape(H, D, HID)

    kk = np.arange(P)[:, None]
    qq = np.arange(P)[None, :]
    tri = (kk <= qq).astype(BF16)                        # [128, 128]
    # trineg[p, c] = -200 where key (col after transpose-by-matmul) > query:
    # lhsT layout -> out[k, q'] = trineg[q', k], want -200 iff k > q'.
    trineg = np.where(kk[:, :] < qq[:, :], np.float32(-200.0),
                      np.float32(0.0)).astype(BF16)      # [q', k] as [p, c]
    xT = np.ascontiguousarray(x.transpose(0, 2, 1)).astype(BF16)  # [B, HID, S]

    def swz(w):  # [HID, N] -> [P, KO*N] so each partition's DMA is contiguous
        n = w.shape[1]
        return np.ascontiguousarray(
            w.reshape(KO, P, n).transpose(1, 0, 2).reshape(P, KO * n))

    in_maps = []
    for c in range(N_CORES):
        b, g = c // 4, c % 4
        wo_g = wo_r[4 * g:4 * g + HQ].astype(BF16)      # [HQ, P, HID]
        in_maps.append({
            "xT": xT[b],
            "wq": swz(wq_p[:, 4 * g:4 * g + HQ, :].reshape(HID, HQ * D)),
            "wk": swz(wk_p[:, g, :]),
            "wv": swz(wv_r[:, g, :]),
            "wo": np.ascontiguousarray(
                wo_g.transpose(1, 0, 2).reshape(P, HQ * HID)),
            "cs2": cs2,
            "ss2": ss2,
            "trineg": trineg,
        })
    return in_maps


def _ensure_ntff_hook():
    """Optional: register the NTFF profiling hook if the image's antenv lacks
    it, so BASS_TRACE=1 produces a profile instead of crashing. No-op on
    failure or when the hook already exists."""
    import sys as _sys
    import types as _types
    try:
        from antenv.axon_hooks import get_axon_ntff_profile_hook  # noqa: F401
        return
    except ImportError:
        pass
    try:
        from trn_agent_boot.trn_boot import _ntff_profile_via_ctypes
        hook = _ntff_profile_via_ctypes("/opt/axon/libaxon_pjrt.so")
        mod = _types.ModuleType("antenv.axon_hooks")
        mod.get_axon_ntff_profile_hook = lambda: hook
        mod.set_axon_ntff_profile_hook = lambda h: None
        _sys.modules["antenv.axon_hooks"] = mod
    except Exception:
        pass


def kernel(x, freqs_cis, wq, wk, wv, wo):
    from concourse.bass_utils import run_bass_kernel_spmd
    _ensure_ntff_hook()

    nc = _CACHE.get("nc")
    if nc is None:
        nc = _build_nc()
        _CACHE["nc"] = nc

    in_maps = _prep_inputs(x, freqs_cis, wq, wk, wv, wo)
    res = run_bass_kernel_spmd(nc, in_maps, list(range(N_CORES)))
    _CACHE["last_result"] = res
    parts = [np.asarray(res.results[c]["out"]).astype(np.float32)
             for c in range(N_CORES)]
    out = np.stack([parts[0] + parts[1] + parts[2] + parts[3],
                    parts[4] + parts[5] + parts[6] + parts[7]])
    return out
